# revision 39
# baseline (speedup 1.0000x reference)
"""Trainium2 Bass kernel for nn_Attention_48799418417201.

Multi-head attention (B=8, S=1024, E=768, H=12, D=64) with LoRA (R=16) on the
QKV projections. Data-parallel over batch: one batch element per NeuronCore,
8 cores. Measured ~181.6-184us/core traced (baseline 210.7us traced /
186.0us untraced), rel err 8.8e-3.

Layout strategy (per core):
  - Host folds (all exact): LoRA delta into W; k-bias dropped (constant-in-j
    shifts cancel in softmax); v-bias folded through Wo into ob
    (softmax rows sum to 1); Q side pre-scaled by A = 128*log2(e)/sqrt(D) so
    scores arrive in "128*log2" units.
  - exp is SPLIT across two engines: ScalarE computes exp via ACT
    (scale=ln2/128, bias=-0.75) into fp8e3 tiles; DVE computes a Schraudolph
    fast-exp: int16 = rne(st + B), bitcast to bf16 == 2^(t - C/128); C=5.5
    centers the sawtooth at +/-3.3%, same band as fp8e3 quantization. Both
    paths produce exp(s - 0.75) * (1 + eps).
  - PV stationary slices are padded to 128 columns so FWL (fast weight load)
    engages (NumWeights==128); output rows 65..127 are garbage and ignored.
  - Output projection is TRANSPOSED: out^T = Wo . attn^T with woT stationary
    and OTu moving; bias ob becomes per-partition -> evac is a cheap
    activation-copy with bias. Host transposes back.
  - All ~9MB of inputs prefetch up front on the SP HWDGE ring in strict
    first-use order (woT last). The first ~25us are HBM-bandwidth-bound, so
    ordering decides the ramp; both rings share the 16 SDMA engines, and
    putting bulk inputs on the ACT ring stalls ScalarE's instruction queue
    (which must run the early k-evacs -- that mistake cost ~5us). Tail
    z-chain DMAs ride the ACT ring past the SP backlog, and the t>=4
    normalizes run on the idle GPSIMD queue so the output projection is not
    gated behind the DVE backlog.
  - A post-Tile "LDW surgery" pass deletes the redundant standalone
    LDWEIGHTS before marked matmuls that reuse the stationary already
    resident in the PE array (scores i=1 pairs, out-proj s-chunk pairs),
    verified against the scheduled instruction stream with row-range
    overlap checks.
  - Emission pacing models all three queues (TensorE / ScalarE / DVE).
  - Things measured NOT to help (reverted): routing mid-kernel DMAs via the
    ACT ring (DMA_DIRECT2D occupies the ScalarE instruction queue);
    stp=2/ppsum=3 PSUM rebalance for projection LDW reuse (score-batch
    pipeline tightening cost ~18us); front-loading exp tiles to DVE and
    loosening the tail pacing slack (both cause in-order queue inversions:
    exp ops land ahead of evacs the PV pipeline needs, +25-35us).
"""

import numpy as np
from contextlib import ExitStack

import concourse.bass as bass
import concourse.bacc as bacc
import concourse.tile as tile
from concourse import mybir
from concourse.bass_utils import run_bass_kernel_spmd

P = 128
S = 1024  # sequence length
E = 768  # embedding
H = 12  # heads
D = 64  # head dim
R = 16  # lora rank
NT = E // P  # 6 n-tiles (also e-tiles) per 768-wide dim
MC = S // 512  # 2 moving-chunks of 512 along sequence
MS = S // P  # 8 sequence subtiles of 128
JT = S // P  # 8 j-tiles (key blocks)
IC = S // 512  # 2 i-chunks (query blocks of 512)
VW = D + 1  # 65 real columns per head in V_aug
VA_W = H * VW + 68  # padded so every [*, h*VW : h*VW+128] LDW slice is legal

F16 = mybir.dt.float16
F32 = mybir.dt.float32
BF16 = mybir.dt.bfloat16
I16 = mybir.dt.int16
F8 = mybir.dt.float8e3

LOG2E = float(np.log2(np.e))
A_SCALE = 128.0 * LOG2E  # folded into Q on host (with 1/sqrt(D))
EXP_SHIFT = 0.75  # exp(s - 0.75) keeps fp8e3 under its 15.5 max-normal
SCHRAUD_C = 5.5  # centers the 2^f ~ 1+f sawtooth
SCHRAUD_B = 127.0 * 128.0 - EXP_SHIFT * A_SCALE - SCHRAUD_C

# exp-engine assignment: hh=0 -> ScalarE; hh=1 -> DVE except every 5th batch.
def exp_engine(t, j, hh):
    if hh == 0:
        return "scal"
    return "scal" if ((t * JT + j) % 5 == 4) else "vec"


def build_nc():
    nc = bacc.Bacc("TRN2", target_bir_lowering=False, debug=False, num_devices=8)

    xT = {
        name: nc.dram_tensor(f"x{name}T", [E, S], F16, kind="ExternalInput")
        for name in ("q", "k", "v")
    }
    wT_d = nc.dram_tensor("wT", [E, 3 * E], F16, kind="ExternalInput")
    woT_d = nc.dram_tensor("woT", [E, E], F16, kind="ExternalInput")
    bq_d = nc.dram_tensor("bq", [P, NT], F32, kind="ExternalInput")
    ob_d = nc.dram_tensor("ob", [P, NT], F32, kind="ExternalInput")
    out_d = nc.dram_tensor("out", [E, S], F16, kind="ExternalOutput")

    with tile.TileContext(nc) as tc, ExitStack() as perm:
        pp = perm.enter_context(tc.tile_pool(name="perm", bufs=1))

        QT = [pp.tile([P, S], F16, name=f"QT{t}", tag=f"QT{t}") for t in range(NT)]
        KT = [pp.tile([P, S], F16, name=f"KT{t}", tag=f"KT{t}") for t in range(NT)]
        Va = [pp.tile([P, VA_W], F16, name=f"Va{g}", tag=f"Va{g}") for g in range(MS)]
        OTu = [pp.tile([P, S], F16, name=f"OTu{t}", tag=f"OTu{t}") for t in range(NT)]
        woT = pp.tile([P, NT, E], F16, name="woT", tag="woT")
        bq_sb = pp.tile([P, NT], F32, name="bq_sb", tag="bq_sb")
        ob_sb = pp.tile([P, NT], F32, name="ob_sb", tag="ob_sb")
        zbias = pp.tile([P, 1], F32, name="zbias", tag="zbias")
        warm = pp.tile([P, 512], F16, name="warm", tag="warm")

        nc.vector.memset(zbias[:], -EXP_SHIFT)
        nc.sync.dma_start(bq_sb[:], bq_d.ap()[:])

        # ---------------- pools ----------------
        # PSUM bank budget: ppsum 2 + stp 3x2 = 8 during projections/PV;
        # transposed output projection uses op 3x1 + fp via stp's freed banks.
        wpool = tc.alloc_tile_pool(name="wpool", bufs=1)
        ppsum = tc.alloc_tile_pool(name="ppsum", bufs=2, space="PSUM")
        xp = tc.alloc_tile_pool(name="xp", bufs=4)
        xp2 = tc.alloc_tile_pool(name="xp2", bufs=2)
        stp = tc.alloc_tile_pool(name="stp", bufs=3, space="PSUM")
        ep = tc.alloc_tile_pool(name="ep", bufs=34)
        ep16 = tc.alloc_tile_pool(name="ep16", bufs=18)
        sgp = tc.alloc_tile_pool(name="sgp", bufs=2)
        zbp = tc.alloc_tile_pool(name="zbp", bufs=2)
        zsp = tc.alloc_tile_pool(name="zsp", bufs=1)
        dpool = tc.alloc_tile_pool(name="dpool", bufs=1, space="DRAM")
        wqk = tc.alloc_tile_pool(name="wqk", bufs=1)
        zdram = dpool.tile([H, S], F32, name="zdram", tag="zdram")

        wreg = {
            "q": wqk.tile([P, NT, E], F16, name="wq", tag="wq"),
            "k": wqk.tile([P, NT, E], F16, name="wk", tag="wk"),
            "v": wpool.tile([P, NT, E], F16, name="wv", tag="wv"),
        }

        # ---------------- PE warm-up ----------------
        nc.vector.memset(warm[:], 0.0)
        wps = ppsum.tile([P, 512], F32, name="wps", tag="acc")
        for _ in range(10):
            nc.tensor.matmul(wps[0:P, :], warm[:, 0:P], warm[:], skip_group_check=True)

        # ---------------- weight/x prefetch ----------------
        xcq = [xp.tile([P, NT, 512], F16, name=f"xcq{m}", tag="xc") for m in range(MC)]
        xck = [xp.tile([P, NT, 512], F16, name=f"xck{m}", tag="xc") for m in range(MC)]

        def dma_x(xc, name, m, eng=None, split=0):
            eng = eng or nc.sync
            msl = slice(m * 512, (m + 1) * 512)
            if split:
                kc = NT // split
                for c in range(split):
                    ks = slice(c * kc, (c + 1) * kc)
                    eng.dma_start(
                        xc[m][:, ks, :],
                        xT[name].ap()[c * kc * P : (c + 1) * kc * P, msl].rearrange(
                            "(k p) c -> p k c", p=P
                        ),
                    )
            else:
                eng.dma_start(
                    xc[m][:], xT[name].ap()[:, msl].rearrange("(k p) c -> p k c", p=P)
                )

        def dma_w(name, noff, eng=None, split=0):
            eng = eng or nc.sync
            if split:
                kc = NT // split
                for c in range(split):
                    eng.dma_start(
                        wreg[name][:, c * kc : (c + 1) * kc, :],
                        wT_d.ap()[
                            c * kc * P : (c + 1) * kc * P, noff : noff + E
                        ].rearrange("(k p) n -> p k n", p=P),
                    )
            else:
                eng.dma_start(
                    wreg[name][:],
                    wT_d.ap()[:, noff : noff + E].rearrange("(k p) n -> p k n", p=P),
                )

        # ALL inputs prefetched up front, interleaved across both HWDGE rings
        # (SP + ACT) in first-use order. The two rings share SDMA bandwidth
        # but drain independently, so a long transfer on one ring does not
        # head-of-line-block the other. ScalarE's ring is free this early
        # (its first exp is ~20us in).
        xcv = [xp2.tile([P, NT, 512], F16, name=f"xcv{m}", tag="xcv") for m in range(MC)]
        # ALL bulk inputs on the SP ring in strict first-use order. Both
        # HWDGE rings share the 16 SDMA engines (aggregate bandwidth fixed),
        # so splitting bulk loads across rings gains nothing -- but DMAs on
        # the ACT ring occupy ScalarE's instruction queue and were blocking
        # the early k-projection evacuations (delaying the first score
        # batches by ~15us). Chunks are interleaved need-exact: MM(k) of the
        # first q-group needs wq slice k AND xq0 slice k, so the two tensors'
        # k-chunks alternate.
        def w_chunk(name, noff, c, kc=NT // 2):
            nc.sync.dma_start(
                wreg[name][:, c * kc : (c + 1) * kc, :],
                wT_d.ap()[
                    c * kc * P : (c + 1) * kc * P, noff : noff + E
                ].rearrange("(k p) n -> p k n", p=P),
            )

        def x_chunk(xc, name, m, c, kc=NT // 2):
            msl = slice(m * 512, (m + 1) * 512)
            nc.sync.dma_start(
                xc[m][:, c * kc : (c + 1) * kc, :],
                xT[name].ap()[c * kc * P : (c + 1) * kc * P, msl].rearrange(
                    "(k p) c -> p k c", p=P
                ),
            )

        w_chunk("q", 0, 0)
        x_chunk(xcq, "q", 0, 0)
        w_chunk("q", 0, 1)
        x_chunk(xcq, "q", 0, 1)
        dma_x(xcq, "q", 1, eng=nc.sync)
        w_chunk("k", E, 0)
        x_chunk(xck, "k", 0, 0)
        w_chunk("k", E, 1)
        x_chunk(xck, "k", 0, 1)
        dma_x(xck, "k", 1, eng=nc.sync)
        dma_w("v", 2 * E, eng=nc.sync)
        for m in range(MC):
            dma_x(xcv, "v", m, eng=nc.sync)
        nc.sync.dma_start(woT[:], woT_d.ap().rearrange("(k p) n -> p k n", p=P))

        exps = {}
        # ring-tenant tracking for the two exp pools (for sb_safe)
        ring = {"scal": [], "vec": []}
        RING_N = {"scal": 34, "vec": 18}

        def emit_score_batch(t, j):
            jsl = slice(j * P, (j + 1) * P)
            sts = [
                stp.tile([P, S], F32, name=f"st{t}_{j}_{hh}", tag="st")
                for hh in range(2)
            ]
            for i in range(IC):
                isl = slice(i * 512, (i + 1) * 512)
                for hh in range(2):
                    base = hh * D
                    mm = nc.tensor.matmul(
                        sts[hh][:, isl],
                        KT[t][base : base + D, jsl],
                        QT[t][base : base + D, isl],
                        tile_position=(base, 0),
                    )
                    if i > 0:
                        mark_reuse(mm)
            for hh in range(2):
                eng = exp_engine(t, j, hh)
                if eng == "scal":
                    ex = ep.tile([P, S], F8, name=f"ex{t}_{j}_{hh}", tag="ex")
                    nc.scalar.activation(
                        ex[:], sts[hh][:], mybir.ActivationFunctionType.Exp,
                        bias=zbias[:], scale=float(np.log(2.0) / 128.0),
                    )
                else:
                    ex = ep16.tile([P, S], I16, name=f"ex{t}_{j}_{hh}", tag="ex16")
                    nc.vector.tensor_scalar(
                        ex[:], sts[hh][:], SCHRAUD_B, None, mybir.AluOpType.add
                    )
                ring[eng].append((t, hh, j))
                exps[(t, hh, j)] = (ex, eng)
                tmodel[eng] = max(tmodel[eng], tmodel["est"]) + 1.35

        def emit_qk_group(name, xc, m, n):
            dest = QT if name == "q" else KT
            msl = slice(m * 512, (m + 1) * 512)
            nsl = slice(n * P, (n + 1) * P)
            acc = ppsum.tile([P, 512], F32, name=f"acc_{name}{m}_{n}", tag="acc")
            for k in range(NT):
                nc.tensor.matmul(
                    acc[:], wreg[name][:, k, nsl], xc[:, k, :],
                    start=(k == 0), stop=(k == NT - 1),
                )
            if name == "q":
                nc.vector.tensor_scalar_add(
                    dest[n][:, msl], acc[:], bq_sb[:, n : n + 1]
                )
                tmodel["vec"] = max(tmodel["vec"], tmodel["est"]) + 0.9
            else:
                nc.scalar.activation(
                    dest[n][:, msl], acc[:], mybir.ActivationFunctionType.Copy
                )
                tmodel["scal"] = max(tmodel["scal"], tmodel["est"]) + 0.75

        def emit_v_setup():
            for g in range(MS):
                va_cols = Va[g][:, 0 : H * VW].rearrange("p (h c) -> p h c", c=VW)
                nc.vector.memset(va_cols[:, :, D], 1.0)
                # pad region beyond the real H*VW columns must be written so
                # the padded 128-wide LDW slices have defined producers
                nc.vector.memset(Va[g][:, H * VW : VA_W], 0.0)

        def emit_proj_v_group(xc, m, ms_i, nch):
            g = m * 4 + ms_i
            va_v = Va[g][:, 0 : H * VW].rearrange("p (h c) -> p h c", c=VW)
            ncols = 512 if nch == 0 else E - 512
            nsl = slice(nch * 512, nch * 512 + ncols)
            acc = ppsum.tile([P, 512], F32, name=f"accv{g}_{nch}", tag="acc")
            for k in range(NT):
                nc.tensor.matmul(
                    acc[:, :ncols],
                    xc[:, k, ms_i * P : (ms_i + 1) * P],
                    wreg["v"][:, k, nsl],
                    start=(k == 0), stop=(k == NT - 1),
                )
            h0 = nch * 8
            nh = 8 if nch == 0 else 4
            acc_v = acc[:, :ncols].rearrange("p (h c) -> p h c", c=D)
            if (g + nch) % 2 == 0:
                nc.vector.tensor_copy(va_v[:, h0 : h0 + nh, 0:D], acc_v[:])
                tmodel["vec"] = max(tmodel["vec"], tmodel["est"]) + 0.85
            else:
                nc.scalar.activation(
                    va_v[:, h0 : h0 + nh, 0:D],
                    acc_v[:],
                    mybir.ActivationFunctionType.Copy,
                )
                tmodel["scal"] = max(tmodel["scal"], tmodel["est"]) + 0.7

        def emit_pv_group(t, i, hh, zt):
            isl = slice(i * 512, (i + 1) * 512)
            h = 2 * t + hh
            base = hh * D
            pv = ppsum.tile([P, 512], F32, name=f"pv{h}_{i}", tag="acc")
            for j in range(JT):
                ex, eng = exps[(t, hh, j)]
                mov = ex.bitcast(BF16) if eng == "vec" else ex
                nc.tensor.matmul(
                    pv[:, :],
                    Va[j][:, h * VW : h * VW + P],
                    mov[:, isl],
                    start=(j == 0), stop=(j == JT - 1),
                )
            stage = sgp.tile([VW, 512], F16, name=f"stg{h}_{i}", tag="stg")
            if t >= 3:
                nc.scalar.activation(
                    stage[:], pv[0:VW, :], mybir.ActivationFunctionType.Copy
                )
                tmodel["scal"] = max(tmodel["scal"], tmodel["est"]) + 0.7
            else:
                nc.vector.tensor_copy(stage[:], pv[0:VW, :])
                tmodel["vec"] = max(tmodel["vec"], tmodel["est"]) + 0.85
            nc.sync.dma_start(OTu[t][base : base + D, isl], stage[0:D, :])
            zq = nc.scalar if t >= 4 else nc.sync
            zq.dma_start(zt[hh : hh + 1, :], stage[D : D + 1, :])

        def emit_pv_zchain(t, i, zb, zt):
            isl = slice(i * 512, (i + 1) * 512)
            z32 = zsp.tile([2, 512], F32, name=f"z32_{t}_{i}", tag="z32")
            rz = zsp.tile([2, 512], F32, name=f"rz{t}_{i}", tag="rz")
            zq = nc.scalar if t >= 4 else nc.sync
            nc.vector.tensor_copy(z32[:], zt[:])
            nc.vector.reciprocal_approx_fast(rz[:], z32[:])
            zq.dma_start(zdram[2 * t : 2 * t + 2, isl], rz[:])
            for hh in range(2):
                zq.dma_start(
                    zb[hh * D : (hh + 1) * D, isl],
                    zdram[2 * t + hh, isl].partition_broadcast(D),
                )

        def emit_pv_norm(t, i, zb):
            isl = slice(i * 512, (i + 1) * 512)
            if t >= 5:
                # the last norms gate the output projection; run them on the
                # otherwise-idle GPSIMD queue so they don't sit behind the
                # DVE backlog (t=4 stays on DVE so the two late norm pairs
                # run on parallel queues)
                nc.gpsimd.tensor_tensor(
                    OTu[t][:, isl], OTu[t][:, isl], zb[:, isl],
                    mybir.AluOpType.mult,
                )
            else:
                nc.vector.tensor_mul(OTu[t][:, isl], OTu[t][:, isl], zb[:, isl])
                tmodel["vec"] = max(tmodel["vec"], tmodel["est"]) + 0.85

        # ---------------- emission sequence ----------------
        reuse_marks = []

        def mark_reuse(mm):
            reuse_marks.append(mm.ins.name)

        emitted_pv = set()
        sb_queue = [(tau, j) for tau in range(NT) for j in range(JT)]
        tmodel = {"est": 0.0, "scal": 0.0, "vec": 0.0}

        def sb_slack():
            return 2.6

        def bump(cost):
            tmodel["est"] += cost

        def sb_safe(tau, j):
            for hh in range(2):
                eng = exp_engine(tau, j, hh)
                tenants = ring[eng]
                prev = len(tenants) - RING_N[eng]
                if prev < 0:
                    continue
                pt, phh, pj = tenants[prev]
                if (pt, IC - 1, phh) not in emitted_pv:
                    return False
            return True

        def sb_backlog_ok(tau, j):
            # emit only when each assigned engine's modeled backlog stays
            # within slack of the tensor-engine emission time
            for hh in range(2):
                eng = exp_engine(tau, j, hh)
                if tmodel[eng] > tmodel["est"] + sb_slack():
                    return False
            return True

        def drain_sb(ready_tau):
            while sb_queue:
                tau, j = sb_queue[0]
                if tau > ready_tau or not sb_safe(tau, j):
                    break
                if not sb_backlog_ok(tau, j):
                    break
                sb_queue.pop(0)
                emit_score_batch(tau, j)
                bump(0.50)

        for n in range(NT):
            for name, xcl in (("q", xcq), ("k", xck)):
                for m in range(MC):
                    emit_qk_group(name, xcl[m], m, n)
                    bump(1.55)
                    drain_sb(n - 1)
                if n == 0 and name == "q" and m == MC - 1:
                    wps2 = ppsum.tile([P, 512], F32, name="wps2", tag="acc")
                    for _ in range(8):
                        nc.tensor.matmul(
                            wps2[0:P, :], warm[:, 0:P], warm[:],
                            skip_group_check=True,
                        )
            drain_sb(n)
        wqk.release()
        emit_v_setup()
        for m in range(MC):
            for ms_i in range(4):
                for nch in range(2):
                    emit_proj_v_group(xcv[m], m, ms_i, nch)
                    bump(0.96)
                    drain_sb(NT - 1)
        nc.sync.dma_start(ob_sb[:], ob_d.ap()[:])
        pending_norm = None
        for t in range(NT):
            while sb_queue and sb_queue[0][0] <= t:
                tau, j = sb_queue.pop(0)
                emit_score_batch(tau, j)
            zb = zbp.tile([P, S], F32, name=f"zb{t}", tag="zb")
            for i in range(IC):
                zt = zsp.tile([2, 512], F16, name=f"zt{t}_{i}", tag="zt")
                for hh in range(2):
                    emit_pv_group(t, i, hh, zt)
                    emitted_pv.add((t, i, hh))
                    bump(2.1)
                    drain_sb(NT - 1)
                emit_pv_zchain(t, i, zb, zt)
                if pending_norm is not None:
                    emit_pv_norm(*pending_norm)
                pending_norm = (t, i, zb)
                if t == NT - 1:
                    # no deferral at the tail: the out-projection is gated on
                    # the last OTu normalizations
                    emit_pv_norm(*pending_norm)
                    pending_norm = None
        if pending_norm is not None:
            emit_pv_norm(*pending_norm)
        dpool.release()
        zsp.release()
        zbp.release()
        sgp.release()
        ep16.release()
        ep.release()
        stp.release()
        xp2.release()
        xp.release()
        ppsum.release()
        wpool.release()

        # ---------------- Phase O: transposed output projection ----------------
        # out^T[oc*128 : , s] = sum_e woT[:, e, oc-slice].T @ OTu[e][:, s-slice]
        with ExitStack() as octx:
            op = octx.enter_context(tc.tile_pool(name="op", bufs=4, space="PSUM"))
            fp = octx.enter_context(tc.tile_pool(name="fp", bufs=4))

            for oc in range(NT):
                ocl = slice(oc * P, (oc + 1) * P)
                accs = [
                    op.tile([P, 512], F32, name=f"oacc{oc}_{sc}", tag="oacc")
                    for sc in range(MC)
                ]
                for e in range(NT):
                    for sc in range(MC):
                        ssl = slice(sc * 512, (sc + 1) * 512)
                        mm = nc.tensor.matmul(
                            accs[sc][:],
                            woT[:, e, ocl],
                            OTu[e][:, ssl],
                            start=(e == 0),
                            stop=(e == NT - 1),
                        )
                        if sc > 0:
                            mark_reuse(mm)
                for sc in range(MC):
                    ssl = slice(sc * 512, (sc + 1) * 512)
                    fin = fp.tile([P, 512], F16, name=f"fin{oc}_{sc}", tag="fin")
                    if (oc + sc) % 2 == 0:
                        nc.scalar.activation(
                            fin[:], accs[sc][:],
                            mybir.ActivationFunctionType.Identity,
                            bias=ob_sb[:, oc : oc + 1],
                        )
                    else:
                        nc.vector.tensor_scalar_add(
                            fin[:], accs[sc][:], ob_sb[:, oc : oc + 1]
                        )
                    (nc.sync if (oc + sc) % 2 == 0 else nc.scalar).dma_start(
                        out_d.ap()[ocl, ssl], fin[:]
                    )

    _ldw_surgery(nc, reuse_marks)
    nc.compile()
    return nc


def _ldw_surgery(nc, reuse_marks):
    """Delete the standalone InstLdweights preceding each marked matmul.

    The marked matmuls reuse the stationary operand already resident in the
    PE array (loaded by an earlier identical LDWEIGHTS, with no intervening
    load to an overlapping row group), so the reload only steals array
    cycles. Any waits/updates on the deleted load are moved onto the matmul
    (generate_event_semaphores legalizes >1 waits later in compile).
    """
    marks = set(reuse_marks)
    removed = 0
    for f in nc.m.functions:
        for bb in f.blocks:
            insts = list(bb.instructions)
            kill_idx = set()

            def ld_rows(ld):
                pos = (ld.tile_position or (0, 0))[0]
                size = ld.tile_size
                nrows = size[0] if size else P
                return pos, pos + nrows

            for idx, ins in enumerate(insts):
                if type(ins).__name__ != "InstMatmult" or ins.name not in marks:
                    continue
                w_ap = repr(ins.ins[1])
                row_lo = (ins.tile_position or (0, 0))[0]
                ts = ins.tile_size
                row_hi = row_lo + (ts[0] if ts else P)
                # Walk back over live LDWEIGHTS whose row range overlaps this
                # matmul's rows. The nearest one must be the redundant
                # identical reload (delete it); the next nearest must be the
                # identical original load (the effective loader). Any other
                # overlapping load in between makes the reuse unsafe.
                near_ld = None
                effective_ok = False
                j = idx - 1
                while j >= 0:
                    pj = insts[j]
                    if type(pj).__name__ == "InstLdweights" and j not in kill_idx:
                        lo, hi = ld_rows(pj)
                        if lo < row_hi and row_lo < hi:  # range overlap
                            same = (
                                repr(pj.ins[0]) == w_ap
                                and lo == row_lo
                                and hi == row_hi
                            )
                            if near_ld is None:
                                if not same:
                                    break
                                near_ld = (j, pj)
                            else:
                                effective_ok = same
                                break
                    j -= 1
                if near_ld is None or not effective_ok:
                    continue
                jdx, ld = near_ld
                si = ld.sync_info
                if si is not None and (len(si.on_wait) or len(si.on_update)):
                    mi = ins.sync_info
                    if mi is None:
                        ins.sync_info = mybir.SyncInfo(
                            on_wait=list(si.on_wait), on_update=list(si.on_update)
                        )
                    else:
                        mi.on_wait = list(mi.on_wait) + list(si.on_wait)
                        mi.on_update = list(mi.on_update) + list(si.on_update)
                kill_idx.add(jdx)
            for jdx in sorted(kill_idx, reverse=True):
                del bb.instructions[jdx]
                removed += 1
    nc._ldw_removed = removed


def _prep_inputs(q, k, v, in_proj_weight, in_proj_bias, out_w, out_b, lora_a, lora_b):
    scale = float(D) ** -0.5
    q = np.asarray(q, np.float32)
    k = np.asarray(k, np.float32)
    v = np.asarray(v, np.float32)
    in_proj_weight = np.asarray(in_proj_weight, np.float32)
    in_proj_bias = np.asarray(in_proj_bias, np.float32)
    out_w = np.asarray(out_w, np.float32)
    out_b = np.asarray(out_b, np.float32)
    lora_a = np.asarray(lora_a, np.float32)
    lora_b = np.asarray(lora_b, np.float32)

    # Fold the LoRA delta into the base weight exactly (fp32 on host):
    w_eff = in_proj_weight + lora_b @ lora_a  # [3E, E]
    wT = w_eff.T.copy()  # [E, 3E]
    # Q path carries 1/sqrt(D) and the 128*log2e Schraudolph scale.
    wT[:, :E] *= scale * A_SCALE
    bq = (in_proj_bias[:E] * scale * A_SCALE).reshape(NT, P).T  # [P, NT]
    # k-bias dropped (cancels in softmax); v-bias folded into ob.
    ob_eff = out_b + out_w @ in_proj_bias[2 * E :]
    ob = ob_eff.reshape(NT, P).T  # [P, NT] per-partition layout for out^T

    shared = {
        "wT": np.ascontiguousarray(wT, np.float16),
        "woT": np.ascontiguousarray(out_w.T, np.float16),
        "bq": np.ascontiguousarray(bq, np.float32),
        "ob": np.ascontiguousarray(ob, np.float32),
    }
    in_maps = []
    for b in range(8):
        m = dict(shared)
        m["xqT"] = np.ascontiguousarray(q[b].T, np.float16)
        m["xkT"] = np.ascontiguousarray(k[b].T, np.float16)
        m["xvT"] = np.ascontiguousarray(v[b].T, np.float16)
        in_maps.append(m)
    return in_maps


_NC_CACHE = {}


def run(inputs, trace=False, **spmd_kwargs):
    if "nc" not in _NC_CACHE:
        _NC_CACHE["nc"] = build_nc()
    nc = _NC_CACHE["nc"]
    in_maps = _prep_inputs(
        inputs["q"],
        inputs["k"],
        inputs["v"],
        inputs["in_proj_weight"],
        inputs["in_proj_bias"],
        inputs["out_w"],
        inputs["out_b"],
        inputs["lora_a"],
        inputs["lora_b"],
    )
    res = run_bass_kernel_spmd(
        nc, in_maps, core_ids=list(range(8)), trace=trace, **spmd_kwargs
    )
    out = np.stack(
        [res.results[b]["out"].T for b in range(8)]
    ).astype(np.float32)
    return out, res


def kernel(
    q,
    k,
    v,
    in_proj_weight,
    in_proj_bias,
    out_w,
    out_b,
    lora_a,
    lora_b,
    num_heads=12,
    **_unused,
):
    assert int(num_heads) == H
    out, _ = run(
        {
            "q": q,
            "k": k,
            "v": v,
            "in_proj_weight": in_proj_weight,
            "in_proj_bias": in_proj_bias,
            "out_w": out_w,
            "out_b": out_b,
            "lora_a": lora_a,
            "lora_b": lora_b,
        }
    )
    return out
